# revision 25
# baseline (speedup 1.0000x reference)
# FVSBN kernel for Trainium2: out = x @ (W * tril(-1)).T + b
#   x: [65536, 764] f32, W: [764, 764] f32, b: [764] f32 -> out: [65536, 764] f32
#
# Strategy: data-parallel over batch across 8 NeuronCores (8192 rows each).
# On each core out^T = Wm^T-tiles.T @ x^T as a block-lower-triangular matmul:
# output tile row nt only needs contraction tiles dt <= nt (21 of 36 pairs).
#   - stationary (lhsT): Wm^T tile [128 d, 128 n]; moving (rhs): x^T tile
#     [128 d, bb b]; psum [128 n, bb b] accumulates over dt.
#   - per half (4096 cols), nt is the OUTER loop and column-groups inner, so
#     each nt's eviction results merge into one [128, 4096] staging tile and
#     a single out-DMA (12 out DMAs/rep instead of 24 -> less HWDGE serial
#     time).
#   - nt order: natural (0..5) in the first half of a cold run so compute
#     starts as soon as the first x tiles land; interleaved (0,5,1,4,2,3)
#     at steady state so short and long accumulation chains alternate and
#     PSUM-bank turnaround never stalls the PE.
#   - eviction fuses the bias add, round-robined DVE/ACT (Pool can't read
#     PSUM).  x loads ride the SP HWDGE ring; w/bias/out ride the ACT ring.
#   - x tiles triple-buffered in half-sized sets: the load of half k+2 is
#     issued at the start of half k, so steady-state compute never waits.
#   - warm-up matmuls on a memset scratch tile keep the PE busy from ~0.3us
#     (HAM un-throttle needs ~3.4us of sustained activity) while the first
#     x chunks and weights stream in.
# Host gathers by transposing each core's out^T back.

import numpy as np

B = 65536
D = 764
NCORES = 8
BPC = B // NCORES  # 8192 rows per core
P = 128
NT = 6  # ceil(764/128)
DP = NT * P  # 768, zero-padded depth
BB = 512  # matmul moving free dim == psum bank width (fp32)
PAIRS = [(nt, dt) for nt in range(NT) for dt in range(nt + 1)]
PAIR_IDX = {p: j for j, p in enumerate(PAIRS)}
NPAIR = len(PAIRS)  # 21
NT_ORDER = [0, 5, 1, 4, 2, 3]  # alternate short/long chains

# device compute dtypes (np side)
X_DT = "float16"  # dtype of x / W on device
OUT_DT = "float16"  # dtype out^T is written in
MM_DT = "float16"  # dtype the PE sees for the matmul operands


def _np_dt(name):
    import ml_dtypes

    return {
        "float32": np.float32,
        "float16": np.float16,
        "bfloat16": ml_dtypes.bfloat16,
    }[name]


def _build(
    bpc,
    x_dt_str=X_DT,
    out_dt_str=OUT_DT,
    mm_dt_str=MM_DT,
    reps=1,
    ablate_x=False,
    ablate_out=False,
    unroll=8,
    evict="multi",  # "multi" = DVE+ACT round robin, "dve" = DVE only
    grp=4,
    bb=BB,
    nh=2,  # pipeline segments per rep (2 = halves, 4 = quarters)
    lookahead=2,  # segments of x prefetch
    osplit=2,  # out-DMA pieces per (nt, segment)
    xc=4096,  # x-load chunk width
    ew=1,  # psum tiles hold ew*bb cols (evictions are ew*bb wide)
    obufs=3,  # rotating out-staging buffers
    nset=3,
    interleave=True,
    warmup=8,  # warm-up matmuls at kernel start
    loop_warmup=2,  # warm-up matmuls at each unrolled loop-body start
    wsplit=True,  # split the weight DMA per nt-row
    first_natural=True,  # natural nt order in the first cold half
    flat=False,  # reps>1 without For_i (sim-only steady-state analysis)
    hack_same_weight=False,  # timing probe: reuse one weight tile everywhere
    hack_all_start=False,  # timing probe: no PSUM accumulation (start always)
    hack_diag_only=False,  # timing probe: only dt_==nt contraction
    hack_no_evict=False,  # timing probe: matmuls only, no psum eviction
):
    import concourse.mybir as mybir
    from concourse import bacc
    from concourse.tile import TileContext

    x_dt = getattr(mybir.dt, x_dt_str)
    out_dt = getattr(mybir.dt, out_dt_str)
    mm_dt = getattr(mybir.dt, mm_dt_str)
    f32 = mybir.dt.float32

    nc = bacc.Bacc("TRN2", target_bir_lowering=False, debug=False)
    xT = nc.dram_tensor("xt", [DP, bpc], x_dt, kind="ExternalInput")
    wt = nc.dram_tensor("wt", [P, NPAIR * P], x_dt, kind="ExternalInput")
    bias = nc.dram_tensor("bias", [P, NT], f32, kind="ExternalInput")
    outT = nc.dram_tensor("outt", [DP, bpc], out_dt, kind="ExternalOutput")

    def mm(ap):
        return ap if ap.dtype == mm_dt else ap.bitcast(mm_dt)

    NH = nh
    HB = bpc // NH  # cols per segment
    nhb = HB // bb
    GRP = min(grp, nhb)
    OW = GRP * bb  # psum-group width
    NGRP = nhb // GRP  # column-groups per segment
    XC = min(xc or OW, HB)  # x-load chunk width
    NSET = 2 if ablate_x else nset
    LA = min(lookahead, NSET - 1)

    with TileContext(nc) as tc:
        with (
            tc.tile_pool(name="wpool", bufs=1) as wpool,
            tc.tile_pool(name="bpool", bufs=1) as bpool,
            tc.tile_pool(name="xpool", bufs=1) as xpool,
            tc.tile_pool(name="opool", bufs=obufs) as opool,
            tc.tile_pool(
                name="pspool", bufs=8 * 512 // (bb * ew), space="PSUM"
            ) as pspool,
        ):
            bias_sb = bpool.tile([P, NT], f32)
            nc.scalar.dma_start(out=bias_sb, in_=bias.ap())
            w_sb = wpool.tile([P, NPAIR * P], x_dt)
            if wsplit:
                # pairs for output-tile row nt occupy j in [nt(nt+1)/2, +nt+1)
                for r in range(NT):
                    j0 = r * (r + 1) // 2
                    j1 = j0 + r + 1
                    nc.scalar.dma_start(
                        out=w_sb[:, j0 * P : j1 * P],
                        in_=wt.ap()[:, j0 * P : j1 * P],
                    )
            else:
                nc.scalar.dma_start(out=w_sb, in_=wt.ap())

            if warmup or loop_warmup:
                wm = wpool.tile([P, P + bb], x_dt, name="warm")
                nc.vector.memset(wm, 0.0)

            def warm_block(n):
                for _ in range(n):
                    wps = pspool.tile([P, ew * bb], f32, name="ps")
                    nc.tensor.matmul(
                        wps[:, :bb],
                        mm(wm[:, :P]),
                        mm(wm[:, P : P + bb]),
                        start=True,
                        stop=True,
                    )

            xsets = [
                [
                    xpool.tile([P, HB], x_dt, tag=f"xr{s}_{t}", name=f"xr{s}_{t}")
                    for t in range(NT)
                ]
                for s in range(NSET)
            ]

            def load_half(s, half, xcw=None):
                # t-major so tile t is fully resident early (nt-outer compute)
                xcw = xcw or XC
                xset = xsets[s]
                for t in range(NT):
                    for c0 in range(0, HB, xcw):
                        nc.sync.dma_start(
                            out=xset[t][:, c0 : c0 + xcw],
                            in_=xT.ap()[
                                t * P : (t + 1) * P,
                                half * HB + c0 : half * HB + c0 + xcw,
                            ],
                        )

            ev_ctr = [0]

            def evict_block(ps, nt, o_dst, w):
                # GPSIMD/Pool cannot read PSUM on TRN2 -> rotate DVE / ACT
                k = ev_ctr[0] % 2 if evict == "multi" else 0
                ev_ctr[0] += 1
                if k == 1:
                    nc.scalar.activation(
                        out=o_dst,
                        in_=ps,
                        func=mybir.ActivationFunctionType.Identity,
                        bias=bias_sb[:, nt : nt + 1],
                        scale=1.0,
                    )
                else:
                    nc.vector.tensor_add(
                        out=o_dst,
                        in0=ps,
                        in1=bias_sb[:, nt : nt + 1].broadcast_to([P, w]),
                    )

            def compute_half(s, half, first=False):
                xset = xsets[s]
                nt_seq = (
                    list(range(NT))
                    if (first and first_natural)
                    else (NT_ORDER if interleave else list(range(NT)))
                )
                for nt in nt_seq:
                    o_t = None if hack_no_evict else opool.tile(
                        [P, HB], out_dt, name="oblk"
                    )
                    for grp_i in range(NGRP):
                        pss = [
                            pspool.tile([P, ew * bb], f32, name="ps")
                            for _ in range(GRP // ew)
                        ]
                        dts = [nt] if hack_diag_only else list(range(nt + 1))
                        for dt_ in dts:
                            j = 0 if hack_same_weight else PAIR_IDX[(nt, dt_)]
                            for g in range(GRP):
                                c0 = (grp_i * GRP + g) * bb
                                e, eo = divmod(g, ew)
                                nc.tensor.matmul(
                                    pss[e][:, eo * bb : (eo + 1) * bb],
                                    mm(w_sb[:, j * P : (j + 1) * P]),
                                    mm(xset[dt_][:, c0 : c0 + bb]),
                                    start=(hack_all_start or dt_ == dts[0]),
                                    stop=(dt_ == dts[-1]),
                                )
                        if hack_no_evict:
                            continue
                        ww = ew * bb
                        for e in range(GRP // ew):
                            c0 = grp_i * GRP * bb + e * ww
                            evict_block(pss[e], nt, o_t[:, c0 : c0 + ww], ww)
                    if hack_no_evict:
                        continue
                    if ablate_out:
                        nc.scalar.dma_start(
                            out=outT.ap()[nt * P : (nt + 1) * P, 0:8],
                            in_=o_t[:, 0:8],
                        )
                    else:
                        oc = HB // osplit
                        for i in range(osplit):
                            nc.scalar.dma_start(
                                out=outT.ap()[
                                    nt * P : (nt + 1) * P,
                                    half * HB + i * oc : half * HB
                                    + (i + 1) * oc,
                                ],
                                in_=o_t[:, i * oc : (i + 1) * oc],
                            )

            def run_halves(n_segs, first=False):
                # segment k computes on set k%NSET; its load is issued LA
                # segments ahead (WAR deps on set reuse throttle the loads
                # to just-ahead-of-consumption)
                if ablate_x:
                    for k in range(n_segs):
                        compute_half(k % NSET, k % NH, first=(first and k == 0))
                    return
                # cold path: fine-grained prologue loads so the first units
                # start as soon as their chunks land
                xc0 = min(2048, XC) if first else None
                for k in range(min(LA, n_segs)):
                    load_half(k % NSET, k % NH, xcw=xc0)
                for k in range(n_segs):
                    if k + LA < n_segs:
                        load_half((k + LA) % NSET, (k + LA) % NH)
                    compute_half(k % NSET, k % NH, first=(first and k == 0))

            if ablate_x:
                for k in range(min(NSET, NH)):
                    load_half(k % NSET, k % NH)
            if warmup:
                warm_block(warmup)
            if flat and reps > 1:
                run_halves(NH * reps, first=True)
            elif reps == 1:
                run_halves(NH, first=True)
            else:
                assert reps % unroll == 0, (reps, unroll)
                with tc.For_i(
                    0, reps // unroll, 1, hint_engines=(mybir.EngineType.PE,)
                ):
                    if loop_warmup:
                        warm_block(loop_warmup)
                    run_halves(NH * unroll)
    nc.compile()
    _dedup_ldweights(nc, mybir)
    return nc


def _dedup_ldweights(nc, mybir):
    """Remove back-to-back redundant LDWEIGHTS: within a basic block, a
    Ldweights whose weight AP matches the previous PE weight load (with no
    intervening write to that SBUF region and no semaphores attached) leaves
    the PE array state unchanged and can be dropped."""
    n_removed = 0
    for blk in nc.m.functions[0].blocks:
        il = blk.instructions
        last_sig = None
        to_remove = []
        for inst in il:
            if isinstance(inst, mybir.InstLdweights):
                a = inst.ins[0]
                sig = (
                    a.memref,
                    a.offset,
                    str(a.ap),
                    str(a.dtype),
                    bool(inst.is_transpose),
                )
                if (
                    sig == last_sig
                    and not inst.has_wait()
                    and not inst.has_update()
                ):
                    to_remove.append(inst)
                else:
                    last_sig = sig
            elif isinstance(inst, mybir.InstMatmult):
                continue
            else:
                if last_sig is not None:
                    try:
                        outs = inst.outs
                    except AttributeError:
                        outs = []
                    for o in outs or []:
                        if getattr(o, "memref", None) == last_sig[0]:
                            last_sig = None
                            break
        for inst in to_remove:
            il.remove(inst)
        n_removed += len(to_remove)
    return n_removed


def _prep_shared(W, b, x_np_dt):
    # masked transposed weights, packed as the 21 lower-triangular 128x128 tiles
    Wm = W * np.tril(np.ones((D, D), np.float32), k=-1)
    WT = np.zeros((DP, DP), np.float32)
    WT[:D, :D] = Wm.T  # WT[d, n] = Wm[n, d]
    w_packed = np.empty((P, NPAIR, P), x_np_dt)
    for j, (nt, dt_) in enumerate(PAIRS):
        w_packed[:, j, :] = WT[dt_ * P : (dt_ + 1) * P, nt * P : (nt + 1) * P]
    w_packed = np.ascontiguousarray(w_packed.reshape(P, NPAIR * P))
    bias_pad = np.zeros(DP, np.float32)
    bias_pad[:D] = b
    bias_t = np.ascontiguousarray(bias_pad.reshape(NT, P).T)  # [p, t] = b[t*128+p]
    return w_packed, bias_t


def kernel(x, W, b, **build_kw):
    from concourse.bass_utils import run_bass_kernel_spmd

    x_np_dt = _np_dt(X_DT)
    nc = _build(BPC, **build_kw)
    w_packed, bias_t = _prep_shared(W, b, x_np_dt)

    in_maps = []
    for c in range(NCORES):
        xs = x[c * BPC : (c + 1) * BPC]
        xT = np.zeros((DP, BPC), x_np_dt)
        xT[:D] = xs.T
        in_maps.append({"xt": xT, "wt": w_packed, "bias": bias_t})

    res = run_bass_kernel_spmd(nc, in_maps, core_ids=list(range(NCORES)))

    out = np.empty((B, D), np.float32)
    for c in range(NCORES):
        out[c * BPC : (c + 1) * BPC] = (
            res.results[c]["outt"][:D].astype(np.float32).T
        )
    return out


# revision 27
# speedup vs baseline: 1.0174x; 1.0174x over previous
# FVSBN kernel for Trainium2: out = x @ (W * tril(-1)).T + b
#   x: [65536, 764] f32, W: [764, 764] f32, b: [764] f32 -> out: [65536, 764] f32
#
# Strategy: data-parallel over batch across 8 NeuronCores (8192 rows each).
# On each core out^T = Wm^T-tiles.T @ x^T as a block-lower-triangular matmul:
# output tile row nt only needs contraction tiles dt <= nt (21 of 36 pairs).
#   - stationary (lhsT): Wm^T tile [128 d, 128 n]; moving (rhs): x^T tile
#     [128 d, bb b]; psum [128 n, bb b] accumulates over dt.
#   - per half (4096 cols), nt is the OUTER loop and column-groups inner, so
#     each nt's eviction results merge into one [128, 4096] staging tile and
#     a single out-DMA (12 out DMAs/rep instead of 24 -> less HWDGE serial
#     time).
#   - nt order: natural (0..5) in the first half of a cold run so compute
#     starts as soon as the first x tiles land; interleaved (0,5,1,4,2,3)
#     at steady state so short and long accumulation chains alternate and
#     PSUM-bank turnaround never stalls the PE.
#   - eviction fuses the bias add, round-robined DVE/ACT (Pool can't read
#     PSUM).  x loads ride the SP HWDGE ring; w/bias/out ride the ACT ring.
#   - x tiles triple-buffered in half-sized sets: the load of half k+2 is
#     issued at the start of half k, so steady-state compute never waits.
#   - warm-up matmuls on a memset scratch tile keep the PE busy from ~0.3us
#     (HAM un-throttle needs ~3.4us of sustained activity) while the first
#     x chunks and weights stream in.
# Host gathers by transposing each core's out^T back.

import numpy as np

B = 65536
D = 764
NCORES = 8
BPC = B // NCORES  # 8192 rows per core
P = 128
NT = 6  # ceil(764/128)
DP = NT * P  # 768, zero-padded depth
BB = 512  # matmul moving free dim == psum bank width (fp32)
PAIRS = [(nt, dt) for nt in range(NT) for dt in range(nt + 1)]
PAIR_IDX = {p: j for j, p in enumerate(PAIRS)}
NPAIR = len(PAIRS)  # 21
NT_ORDER = [0, 5, 1, 4, 2, 3]  # alternate short/long chains

# device compute dtypes (np side)
X_DT = "float16"  # dtype of x / W on device
OUT_DT = "float16"  # dtype out^T is written in
MM_DT = "float16"  # dtype the PE sees for the matmul operands


def _np_dt(name):
    import ml_dtypes

    return {
        "float32": np.float32,
        "float16": np.float16,
        "bfloat16": ml_dtypes.bfloat16,
    }[name]


def _build(
    bpc,
    x_dt_str=X_DT,
    out_dt_str=OUT_DT,
    mm_dt_str=MM_DT,
    reps=1,
    ablate_x=False,
    ablate_out=False,
    unroll=8,
    evict="multi",  # "multi" = DVE+ACT round robin, "dve" = DVE only
    grp=4,
    bb=BB,
    nh=2,  # pipeline segments per rep (2 = halves, 4 = quarters)
    lookahead=2,  # segments of x prefetch
    osplit=2,  # out-DMA pieces per (nt, segment)
    xc=4096,  # x-load chunk width
    ew=1,  # psum tiles hold ew*bb cols (evictions are ew*bb wide)
    obufs=3,  # rotating out-staging buffers
    oring="act",  # HWDGE ring for out stores: "act" or "sp"
    nset=3,
    interleave=True,
    warmup=8,  # warm-up matmuls at kernel start
    loop_warmup=2,  # warm-up matmuls at each unrolled loop-body start
    wsplit=True,  # split the weight DMA per nt-row
    first_natural=True,  # natural nt order in the first cold half
    flat=False,  # reps>1 without For_i (sim-only steady-state analysis)
    hack_same_weight=False,  # timing probe: reuse one weight tile everywhere
    hack_all_start=False,  # timing probe: no PSUM accumulation (start always)
    hack_diag_only=False,  # timing probe: only dt_==nt contraction
    hack_no_evict=False,  # timing probe: matmuls only, no psum eviction
):
    import concourse.mybir as mybir
    from concourse import bacc
    from concourse.tile import TileContext

    x_dt = getattr(mybir.dt, x_dt_str)
    out_dt = getattr(mybir.dt, out_dt_str)
    mm_dt = getattr(mybir.dt, mm_dt_str)
    f32 = mybir.dt.float32

    nc = bacc.Bacc("TRN2", target_bir_lowering=False, debug=False)
    xT = nc.dram_tensor("xt", [DP, bpc], x_dt, kind="ExternalInput")
    wt = nc.dram_tensor("wt", [P, NPAIR * P], x_dt, kind="ExternalInput")
    bias = nc.dram_tensor("bias", [P, NT], f32, kind="ExternalInput")
    outT = nc.dram_tensor("outt", [DP, bpc], out_dt, kind="ExternalOutput")

    def mm(ap):
        return ap if ap.dtype == mm_dt else ap.bitcast(mm_dt)

    NH = nh
    HB = bpc // NH  # cols per segment
    nhb = HB // bb
    GRP = min(grp, nhb)
    OW = GRP * bb  # psum-group width
    NGRP = nhb // GRP  # column-groups per segment
    XC = min(xc or OW, HB)  # x-load chunk width
    NSET = 2 if ablate_x else nset
    LA = min(lookahead, NSET - 1)

    with TileContext(nc) as tc:
        with (
            tc.tile_pool(name="wpool", bufs=1) as wpool,
            tc.tile_pool(name="bpool", bufs=1) as bpool,
            tc.tile_pool(name="xpool", bufs=1) as xpool,
            tc.tile_pool(name="opool", bufs=obufs) as opool,
            tc.tile_pool(
                name="pspool", bufs=8 * 512 // (bb * ew), space="PSUM"
            ) as pspool,
        ):
            bias_sb = bpool.tile([P, NT], f32)
            nc.scalar.dma_start(out=bias_sb, in_=bias.ap())
            w_sb = wpool.tile([P, NPAIR * P], x_dt)
            if wsplit:
                # pairs for output-tile row nt occupy j in [nt(nt+1)/2, +nt+1)
                for r in range(NT):
                    j0 = r * (r + 1) // 2
                    j1 = j0 + r + 1
                    nc.scalar.dma_start(
                        out=w_sb[:, j0 * P : j1 * P],
                        in_=wt.ap()[:, j0 * P : j1 * P],
                    )
            else:
                nc.scalar.dma_start(out=w_sb, in_=wt.ap())

            if warmup or loop_warmup:
                wm = wpool.tile([P, P + bb], x_dt, name="warm")
                nc.vector.memset(wm, 0.0)

            def warm_block(n):
                for _ in range(n):
                    wps = pspool.tile([P, ew * bb], f32, name="ps")
                    nc.tensor.matmul(
                        wps[:, :bb],
                        mm(wm[:, :P]),
                        mm(wm[:, P : P + bb]),
                        start=True,
                        stop=True,
                    )

            xsets = [
                [
                    xpool.tile([P, HB], x_dt, tag=f"xr{s}_{t}", name=f"xr{s}_{t}")
                    for t in range(NT)
                ]
                for s in range(NSET)
            ]

            def load_half(s, half, xcw=None):
                # t-major so tile t is fully resident early (nt-outer compute)
                xcw = xcw or XC
                xset = xsets[s]
                for t in range(NT):
                    for c0 in range(0, HB, xcw):
                        nc.sync.dma_start(
                            out=xset[t][:, c0 : c0 + xcw],
                            in_=xT.ap()[
                                t * P : (t + 1) * P,
                                half * HB + c0 : half * HB + c0 + xcw,
                            ],
                        )

            ev_ctr = [0]

            def evict_block(ps, nt, o_dst, w):
                # GPSIMD/Pool cannot read PSUM on TRN2 -> rotate DVE / ACT
                k = ev_ctr[0] % 2 if evict == "multi" else 0
                ev_ctr[0] += 1
                if k == 1:
                    nc.scalar.activation(
                        out=o_dst,
                        in_=ps,
                        func=mybir.ActivationFunctionType.Identity,
                        bias=bias_sb[:, nt : nt + 1],
                        scale=1.0,
                    )
                else:
                    nc.vector.tensor_add(
                        out=o_dst,
                        in0=ps,
                        in1=bias_sb[:, nt : nt + 1].broadcast_to([P, w]),
                    )

            def compute_half(s, half, first=False):
                xset = xsets[s]
                nt_seq = (
                    list(range(NT))
                    if (first and first_natural)
                    else (NT_ORDER if interleave else list(range(NT)))
                )
                for nt in nt_seq:
                    o_t = None if hack_no_evict else opool.tile(
                        [P, HB], out_dt, name="oblk"
                    )
                    for grp_i in range(NGRP):
                        pss = [
                            pspool.tile([P, ew * bb], f32, name="ps")
                            for _ in range(GRP // ew)
                        ]
                        dts = [nt] if hack_diag_only else list(range(nt + 1))
                        for dt_ in dts:
                            j = 0 if hack_same_weight else PAIR_IDX[(nt, dt_)]
                            for g in range(GRP):
                                c0 = (grp_i * GRP + g) * bb
                                e, eo = divmod(g, ew)
                                nc.tensor.matmul(
                                    pss[e][:, eo * bb : (eo + 1) * bb],
                                    mm(w_sb[:, j * P : (j + 1) * P]),
                                    mm(xset[dt_][:, c0 : c0 + bb]),
                                    start=(hack_all_start or dt_ == dts[0]),
                                    stop=(dt_ == dts[-1]),
                                )
                        if hack_no_evict:
                            continue
                        ww = ew * bb
                        for e in range(GRP // ew):
                            c0 = grp_i * GRP * bb + e * ww
                            evict_block(pss[e], nt, o_t[:, c0 : c0 + ww], ww)
                    if hack_no_evict:
                        continue
                    oeng = nc.sync if oring == "sp" else nc.scalar
                    if ablate_out:
                        oeng.dma_start(
                            out=outT.ap()[nt * P : (nt + 1) * P, 0:8],
                            in_=o_t[:, 0:8],
                        )
                    else:
                        oc = HB // osplit
                        for i in range(osplit):
                            oeng.dma_start(
                                out=outT.ap()[
                                    nt * P : (nt + 1) * P,
                                    half * HB + i * oc : half * HB
                                    + (i + 1) * oc,
                                ],
                                in_=o_t[:, i * oc : (i + 1) * oc],
                            )

            def run_halves(n_segs, first=False):
                # segment k computes on set k%NSET; its load is issued LA
                # segments ahead (WAR deps on set reuse throttle the loads
                # to just-ahead-of-consumption)
                if ablate_x:
                    for k in range(n_segs):
                        compute_half(k % NSET, k % NH, first=(first and k == 0))
                    return
                # cold path: fine-grained prologue loads so the first units
                # start as soon as their chunks land
                xc0 = min(2048, XC) if first else None
                for k in range(min(LA, n_segs)):
                    load_half(k % NSET, k % NH, xcw=xc0)
                for k in range(n_segs):
                    if k + LA < n_segs:
                        load_half((k + LA) % NSET, (k + LA) % NH)
                    compute_half(k % NSET, k % NH, first=(first and k == 0))

            if ablate_x:
                for k in range(min(NSET, NH)):
                    load_half(k % NSET, k % NH)
            if warmup:
                warm_block(warmup)
            if flat and reps > 1:
                run_halves(NH * reps, first=True)
            elif reps == 1:
                run_halves(NH, first=True)
            else:
                assert reps % unroll == 0, (reps, unroll)
                with tc.For_i(
                    0, reps // unroll, 1, hint_engines=(mybir.EngineType.PE,)
                ):
                    if loop_warmup:
                        warm_block(loop_warmup)
                    run_halves(NH * unroll)
    nc.compile()
    _dedup_ldweights(nc, mybir)
    return nc


def _dedup_ldweights(nc, mybir):
    """Remove back-to-back redundant LDWEIGHTS: within a basic block, a
    Ldweights whose weight AP matches the previous PE weight load (with no
    intervening write to that SBUF region and no semaphores attached) leaves
    the PE array state unchanged and can be dropped."""
    n_removed = 0
    for blk in nc.m.functions[0].blocks:
        il = blk.instructions
        last_sig = None
        to_remove = []
        for inst in il:
            if isinstance(inst, mybir.InstLdweights):
                a = inst.ins[0]
                sig = (
                    a.memref,
                    a.offset,
                    str(a.ap),
                    str(a.dtype),
                    bool(inst.is_transpose),
                )
                if (
                    sig == last_sig
                    and not inst.has_wait()
                    and not inst.has_update()
                ):
                    to_remove.append(inst)
                else:
                    last_sig = sig
            elif isinstance(inst, mybir.InstMatmult):
                continue
            else:
                if last_sig is not None:
                    try:
                        outs = inst.outs
                    except AttributeError:
                        outs = []
                    for o in outs or []:
                        if getattr(o, "memref", None) == last_sig[0]:
                            last_sig = None
                            break
        for inst in to_remove:
            il.remove(inst)
        n_removed += len(to_remove)
    return n_removed


def _prep_shared(W, b, x_np_dt):
    # masked transposed weights, packed as the 21 lower-triangular 128x128 tiles
    Wm = W * np.tril(np.ones((D, D), np.float32), k=-1)
    WT = np.zeros((DP, DP), np.float32)
    WT[:D, :D] = Wm.T  # WT[d, n] = Wm[n, d]
    w_packed = np.empty((P, NPAIR, P), x_np_dt)
    for j, (nt, dt_) in enumerate(PAIRS):
        w_packed[:, j, :] = WT[dt_ * P : (dt_ + 1) * P, nt * P : (nt + 1) * P]
    w_packed = np.ascontiguousarray(w_packed.reshape(P, NPAIR * P))
    bias_pad = np.zeros(DP, np.float32)
    bias_pad[:D] = b
    bias_t = np.ascontiguousarray(bias_pad.reshape(NT, P).T)  # [p, t] = b[t*128+p]
    return w_packed, bias_t


def kernel(x, W, b, **build_kw):
    from concourse.bass_utils import run_bass_kernel_spmd

    x_np_dt = _np_dt(X_DT)
    nc = _build(BPC, **build_kw)
    w_packed, bias_t = _prep_shared(W, b, x_np_dt)

    in_maps = []
    for c in range(NCORES):
        xs = x[c * BPC : (c + 1) * BPC]
        xT = np.zeros((DP, BPC), x_np_dt)
        xT[:D] = xs.T
        in_maps.append({"xt": xT, "wt": w_packed, "bias": bias_t})

    res = run_bass_kernel_spmd(nc, in_maps, core_ids=list(range(NCORES)))

    out = np.empty((B, D), np.float32)
    for c in range(NCORES):
        out[c * BPC : (c + 1) * BPC] = (
            res.results[c]["outt"][:D].astype(np.float32).T
        )
    return out
